# revision 5
# baseline (speedup 1.0000x reference)
"""Trainium2 Bass kernel for nn_AngleTripletGenerator (DimeNet-style triplet
generation), distributed over 8 NeuronCores.

Strategy: data-parallel over center nodes (6250/core, padded to 6272 = one
supertile of 128 partitions x 49 nodes).  The angle/distance/mask grids are
symmetric in (j, k), so the device computes only the packed half-grid
H[n, d, j] for d = 1..8 with k = (j + d) mod 16 (j innermost so every grid
operand is step-1 on its innermost axis -> DVE 16-bit 2x perf mode; the
odd-d half reads at a 2-byte-misaligned base, which costs only ~11%).
The mod-16 wraparound is handled by host-extended per-edge tiles of width
24 = 16+8 for x/y/z (and on-chip d2), concatenated channel-major so one
fused TENSOR_TENSOR computes all four (j,k)-products per parity.

All grid math is fp16 on the DVE at 2x mode.  Per-edge d2 is clamped to 100
(valid edges have d2 <= 25) so d2j*d2k fits fp16; x/y/z are prescaled by -2
into an "A" concat so that

  G2 = -2*sum_c xc_j*xc_k = -2G          (plain TT adds, no 1x STT)
  T1 = Square(0.5*G2) = G^2              (ACT, free scale)
  cn2 = T2 - T1,  ry = AbsRsqrt(4*cn2) = 1/(2*sqrt|cn2|)
  t = G2*ry = -G/sqrt(cn2)
  theta = pi/2 + Arctan(t) = atan2(sqrt(cn2), G)   (exact atan2 for y>=0)
  dsq = (d2j + d2k) + G2,  dist = Sqrt(dsq)

AbsRsqrt(0) is a large finite value (3.4e38), so t never becomes NaN;
Arctan handles +-inf (probed).  Masked slots may carry garbage/NaN values:
the host applies np.where(mask, ...) during the half-grid -> full-grid
scatter it performs anyway.  Host also patches two degenerate classes the
reference defines specially: duplicate-neighbor slots (ref distance quirk
1.0, ref angle 0) and self-edge slots (atan2(0,0) = 0), both identified
from edge_index alone.

Host side does layout-only work: the pos gather (indirect DMA can't do it
efficiently), padding/transposes, the half-grid -> full-grid scatter (a
fixed permutation), masked-slot selection, and the id3 outputs, which are
pure broadcasts of edge_index / arange.

The module patches the activation-table catalog so Square resolves to
abs_reciprocal_sqrt_and_small (hiding it elsewhere), giving exactly three
ACT_TABLE_LOADs: absrsqrt-set, sqrt-set, sigmoid-set (Arctan).
"""

import sys

sys.path.insert(0, "/opt/trn_rl_repo")

import numpy as np

import concourse.bass as bass
import concourse.bacc as bacc
import concourse.mybir as mybir
import concourse.tile as tile_mod
import concourse.hw_specs as _hw_specs


def _tables_pin_square(arch):
    """Hide Square outside abs_reciprocal_sqrt_and_small so the kernel's
    first Square pulls in the set AbsRsqrt needs anyway (3 loads total)."""
    t = dict(_hw_specs.get_activation_tables(arch))
    keep = "abs_reciprocal_sqrt_and_small"
    if keep in t:
        for name in list(t):
            if name == keep:
                continue
            sq = [f for f in t[name] if f.name == "Square"]
            if sq:
                t[name] = t[name] - set(sq)
    return t


bacc.get_activation_tables = _tables_pin_square

F32 = mybir.dt.float32
FP16 = mybir.dt.float16
U8 = mybir.dt.uint8

N_NODES = 50000
DEG = 16
ND = 8               # half-grid depth: d = 1..8, k = (j+d) mod 16
GW = DEG * ND        # 128 grid elems per node
EXT = DEG + ND       # 24: extended per-edge tiles for the mod-16 wrap
N_CORES = 8
NPC = N_NODES // N_CORES   # 6250
P = 128
B = 49               # nodes per partition (single supertile)
NPC_PAD = P * B      # 6272
BEXT = B * EXT       # 1176
BGW = B * GW         # 6272
CUTOFF2 = 25.0
D2CLAMP = 100.0      # invalid-edge d2 clamp: T2 <= 1e4 stays in fp16 range
GBIAS = 1e-4         # makes atan2(0,0) slots resolve via the inf path
PI = float(np.pi)

A = mybir.AluOpType
AF = mybir.ActivationFunctionType


def _ap(tile, offset, dims):
    """Free-dim AP on an SBUF tile: dims = [[stride, size], ...] (elements)."""
    base = tile[:]
    return bass.AP(base.tensor, base.offset + offset, [list(base.ap[0])] + dims)


def build_nc():
    nc = bacc.Bacc(None, target_bir_lowering=False, debug=False)

    # host layout: gath row p = [3ch, B, EXT] f32 (neighbor coords, host-
    # extended with the mod-16 wrap); cpt row p = [3ch, B] f32 (center).
    gath_d = nc.dram_tensor("gath", [P, 3 * BEXT], F32, kind="ExternalInput")
    cpt_d = nc.dram_tensor("cpt", [P, 3 * B], F32, kind="ExternalInput")
    phd = nc.dram_tensor("phd", [P, BGW], FP16, kind="ExternalOutput")
    pha = nc.dram_tensor("pha", [P, BGW], FP16, kind="ExternalOutput")
    phm = nc.dram_tensor("phm", [P, BGW], U8, kind="ExternalOutput")

    gath_cv = gath_d[:].rearrange("p (c f) -> c p f", c=3)
    phd_hv = phd[:].rearrange("p (h f) -> h p f", h=2)
    pha_hv = pha[:].rearrange("p (h f) -> h p f", h=2)

    TT = nc.vector.tensor_tensor
    TS = nc.vector.tensor_scalar
    ACT = nc.scalar.activation

    with tile_mod.TileContext(nc) as tc:
        with tc.tile_pool(name="work", bufs=1) as pool:
            cpt = pool.tile([P, 3 * B], F32, tag="cpt")
            gath = pool.tile([P, 3 * BEXT], F32, tag="gath")
            xyzf = pool.tile([P, 3 * BEXT], F32, tag="xyzf")
            sq = pool.tile([P, 3 * BEXT], F32, tag="sq")
            d2f = pool.tile([P, BEXT], F32, tag="d2f")
            pc = pool.tile([P, 4 * BEXT], FP16, tag="pc")    # x|y|z|d2c
            ac = pool.tile([P, 4 * BEXT], FP16, tag="ac")    # -2x|-2y|-2z|d2c
            ve = pool.tile([P, BEXT], FP16, tag="ve")
            pr = pool.tile([P, 4 * BGW], FP16, tag="pr")     # products
            g2 = pool.tile([P, BGW], FP16, tag="g2")
            t1 = pool.tile([P, BGW], FP16, tag="t1")         # T1 -> t -> a
            cn = pool.tile([P, BGW], FP16, tag="cn")         # cn2 -> ry
            t3 = pool.tile([P, BGW], FP16, tag="t3")         # T3 -> dsq -> dist
            m16 = pool.tile([P, BGW], FP16, tag="m16")

            # ---- edge stage (f32 for exact cutoff masking; fp16 casts
            # feed the grid) ----
            nc.sync.dma_start(out=cpt[:], in_=cpt_d[:])
            for ci in range(3):
                nc.sync.dma_start(
                    out=gath[:, ci * BEXT:(ci + 1) * BEXT], in_=gath_cv[ci]
                )
                # x_c = gath_c - cpt_c (broadcast over EXT), f32
                TT(
                    out=_ap(xyzf, ci * BEXT, [[EXT, B], [1, EXT]]),
                    in0=_ap(gath, ci * BEXT, [[EXT, B], [1, EXT]]),
                    in1=_ap(cpt, ci * B, [[1, B], [0, EXT]]),
                    op=A.subtract,
                )
            # fp16 cast on ACT (flat rate, ACT has headroom)
            ACT(out=pc[:, :3 * BEXT], in_=xyzf[:], func=AF.Copy)
            # A-side: -2x|-2y|-2z (one TS over the 3-channel concat)
            TS(out=ac[:, :3 * BEXT], in0=pc[:, :3 * BEXT], scalar1=-2.0,
               scalar2=None, op0=A.mult)
            # d2 = x^2 + y^2 + z^2 in f32 (squares on ACT; adds on DVE)
            ACT(out=sq[:], in_=xyzf[:], func=AF.Square)
            TT(out=d2f[:], in0=sq[:, :BEXT], in1=sq[:, BEXT:2 * BEXT],
               op=A.add)
            TT(out=d2f[:], in0=d2f[:], in1=sq[:, 2 * BEXT:], op=A.add)
            # exact cutoff test on f32 d2; fp16 clamped casts for the grid
            TS(out=ve[:], in0=d2f[:], scalar1=CUTOFF2, scalar2=None,
               op0=A.is_le)
            TS(out=ac[:, 3 * BEXT:], in0=d2f[:], scalar1=D2CLAMP,
               scalar2=None, op0=A.min)
            TS(out=pc[:, 3 * BEXT:], in0=d2f[:], scalar1=D2CLAMP,
               scalar2=None, op0=A.min)

            # ---- grid stage ----
            # fused 4-channel products, split by d-parity for 4B alignment:
            #   pr[ch][b][d-1][j] = A_ch[b][j] * PC_ch[b][j+d]
            def pair(out_t, in0_t, in0_off, in1_t, in1_off, op, nch, width):
                for par in (0, 1):  # 0: even d (aligned), 1: odd d
                    base_k = 2 - par          # d = 2,4,6,8 / 1,3,5,7
                    base_o = (1 - par) * DEG  # rows d-1 = 1,3,5,7 / 0,2,4,6
                    TT(
                        out=_ap(out_t, base_o,
                                [[GW, nch * B], [2 * DEG, 4], [1, DEG]]),
                        in0=_ap(in0_t, in0_off,
                                [[EXT, nch * B], [0, 4], [1, DEG]]),
                        in1=_ap(in1_t, in1_off + base_k,
                                [[EXT, nch * B], [2, 4], [1, DEG]]),
                        op=op,
                    )

            pair(pr, ac, 0, pc, 0, A.mult, 4, DEG)
            # G2 = -2G = prx + pry + prz  (plain fp16 TT adds, 2x mode)
            TT(out=g2[:], in0=pr[:, 0 * BGW:1 * BGW],
               in1=pr[:, 1 * BGW:2 * BGW], op=A.add)
            TT(out=g2[:], in0=g2[:], in1=pr[:, 2 * BGW:3 * BGW], op=A.add)
            # T1 = (0.5*G2)^2 = G^2  (ACT; DVE covers the wait with T3)
            ACT(out=t1[:], in_=g2[:], func=AF.Square, scale=0.5)
            # T3 = d2j + d2k
            pair(t3, pc, 3 * BEXT, pc, 3 * BEXT, A.add, 1, DEG)
            # cn2 = T2 - T1 -> ry = 1/(2*sqrt|cn2|) as early as possible
            TT(out=cn[:], in0=pr[:, 3 * BGW:4 * BGW], in1=t1[:],
               op=A.subtract)
            ACT(out=cn[:], in_=cn[:], func=AF.Abs_reciprocal_sqrt, scale=4.0)
            # dsq = T3 + G2 ; mask pair (both fill the ry wait)
            TT(out=t3[:], in0=t3[:], in1=g2[:], op=A.add)
            pair(m16, ve, 0, ve, 0, A.mult, 1, DEG)
            nc.gpsimd.dma_start(out=phm[:], in_=m16[:])  # fp16 -> u8
            # t = G2 * ry = -G/sqrt(cn2)
            TT(out=t1[:], in0=g2[:], in1=cn[:], op=A.mult)
            # dist = sqrt(dsq) ; theta = pi/2 + atan(t)
            for h in range(2):
                hs = slice(h * BGW // 2, (h + 1) * BGW // 2)
                ACT(out=t3[:, hs], in_=t3[:, hs], func=AF.Sqrt)
                nc.sync.dma_start(out=phd_hv[h], in_=t3[:, hs])
            for h in range(2):
                hs = slice(h * BGW // 2, (h + 1) * BGW // 2)
                ACT(out=t1[:, hs], in_=t1[:, hs], func=AF.Arctan)
                TS(out=t1[:, hs], in0=t1[:, hs], scalar1=PI / 2, scalar2=None,
                   op0=A.add)
                nc.sync.dma_start(out=pha_hv[h], in_=t1[:, hs])

    return nc


_NC_CACHE = {}


def _get_nc():
    if "nc" not in _NC_CACHE:
        nc = build_nc()
        nc.finalize()
        _NC_CACHE["nc"] = nc
    return _NC_CACHE["nc"]


# half-grid [d-1, j] -> full-grid (j, k) scatter indices (fixed permutation)
_JF = np.broadcast_to(np.arange(DEG, dtype=np.int64)[None, :], (ND, DEG))
_KF = (np.arange(DEG, dtype=np.int64)[None, :]
       + np.arange(1, ND + 1, dtype=np.int64)[:, None]) % DEG

_OI_CACHE = {}


def _shard_inputs(pos, col2d):
    in_maps = []
    for c in range(N_CORES):
        lo = c * NPC
        colp = np.zeros((NPC_PAD, DEG), dtype=np.int64)
        colp[:NPC] = col2d[lo:lo + NPC]
        gpv = pos[colp]                                   # [6272, 16, 3]
        ge = np.concatenate([gpv, gpv[:, :ND]], axis=1)   # [6272, 24, 3]
        # -> [P, 3, B, EXT] -> [P, 3*B*EXT]
        ge = ge.reshape(P, B, EXT, 3).transpose(0, 3, 1, 2)
        ge = np.ascontiguousarray(ge).reshape(P, 3 * BEXT)
        cp = np.zeros((NPC_PAD, 3), dtype=np.float32)
        cp[:NPC] = pos[lo:lo + NPC]
        cp = cp.reshape(P, B, 3).transpose(0, 2, 1)
        cp = np.ascontiguousarray(cp).reshape(P, 3 * B)
        in_maps.append({"gath": ge, "cpt": cp})
    return in_maps


def kernel(pos, edge_index, _trace=False):
    """Full-input / full-output entry point. Returns the same tuple as
    reference(): (id3_i, id3_j, id3_k, distances_jk, angles, mask)."""
    from concourse.bass_utils import run_bass_kernel_spmd

    pos = np.asarray(pos, dtype=np.float32)
    edge_index = np.asarray(edge_index, dtype=np.int32)
    n = pos.shape[0]
    deg = edge_index.shape[1] // n
    assert n == N_NODES and deg == DEG

    col2d = edge_index[1].reshape(n, deg)

    nc = _get_nc()
    in_maps = _shard_inputs(pos, col2d)
    res = run_bass_kernel_spmd(
        nc, in_maps, core_ids=list(range(N_CORES)), trace=_trace
    )

    od = np.zeros((n, DEG, DEG), dtype=np.float32)
    oa = np.zeros((n, DEG, DEG), dtype=np.float32)
    om = np.zeros((n, DEG, DEG), dtype=bool)
    arange_n = np.arange(n, dtype=np.int64)
    for c in range(N_CORES):
        lo = c * NPC
        r = res.results[c]
        hd = np.asarray(r["phd"]).reshape(NPC_PAD, ND, DEG)[:NPC]
        ha = np.asarray(r["pha"]).reshape(NPC_PAD, ND, DEG)[:NPC]
        hm = np.asarray(r["phm"]).reshape(NPC_PAD, ND, DEG)[:NPC] != 0
        colc = col2d[lo:lo + NPC].astype(np.int64)
        # degenerate-slot repairs (identified from edge_index/pos alone):
        # duplicate-neighbor slots: ref distance quirk 1.0, ref angle 0
        nb_j = colc[:, _JF]                    # [NPC, ND, DEG] neighbor ids
        nb_k = colc[:, _KF]
        dup = nb_j == nb_k
        # self-edge slots (zero-length edge): ref angle = atan2(0,0) = 0
        selfe = colc == arange_n[lo:lo + NPC, None]
        sz = selfe[:, _JF] | selfe[:, _KF]
        hd = np.where(hm, np.nan_to_num(hd.astype(np.float32), nan=0.0), 0.0)
        ha = np.where(hm, np.nan_to_num(ha.astype(np.float32), nan=0.0), 0.0)
        hd[dup & hm] = 1.0
        ha[(dup | sz) & hm] = 0.0
        sl = slice(lo, lo + NPC)
        od[sl][:, _JF, _KF] = hd
        od[sl][:, _KF, _JF] = hd
        oa[sl][:, _JF, _KF] = ha
        oa[sl][:, _KF, _JF] = ha
        om[sl][:, _JF, _KF] = hm
        om[sl][:, _KF, _JF] = hm

    if "oi" not in _OI_CACHE:
        _OI_CACHE["oi"] = np.repeat(
            np.arange(n, dtype=np.int32), DEG * DEG
        )
    oi = _OI_CACHE["oi"]
    oj = np.ascontiguousarray(
        np.broadcast_to(col2d[:, :, None], (n, DEG, DEG))
    ).reshape(-1)
    ok = np.ascontiguousarray(
        np.broadcast_to(col2d[:, None, :], (n, DEG, DEG))
    ).reshape(-1)

    ret = (oi, oj, ok, od.reshape(-1), oa.reshape(-1), om.reshape(-1))
    if _trace:
        return ret, res
    return ret


# revision 6
# speedup vs baseline: 1.1518x; 1.1518x over previous
"""Trainium2 Bass kernel for nn_AngleTripletGenerator (DimeNet-style triplet
generation), distributed over 8 NeuronCores.

Strategy: data-parallel over center nodes (6250/core, padded to 6272 = one
supertile of 128 partitions x 49 nodes).  The angle/distance/mask grids are
symmetric in (j, k), so the device computes only the packed half-grid
H[n, d, j] for d = 1..8 with k = (j + d) mod 16.  j is the innermost axis,
so every grid operand is step-1 innermost -> DVE 16-bit 2x perf mode; each
k-sourced op is split into an even-d instruction (4-byte-aligned base) and
an odd-d instruction (misaligned base, costs only ~11%).  The mod-16 wrap
is handled by host-extended per-edge tiles of width 24 = 16+8.

All math is fp16.  Per-edge d2 is clamped to 100 (valid edges have
d2 <= 25) so d2j*d2k fits fp16; x/y/z are prescaled by -2 so

  G2 = sum_c (-2 xc_j) * xc_k = -2G       (plain 2x TT adds, no 1x STT)
  T1 = Square(0.5*G2) = G^2               (ACT, free input scale)
  cn2 = T2 - T1,  ry = AbsRsqrt(4*cn2) = 1/(2*sqrt|cn2|)
  t = G2*ry = -G/sqrt(cn2)
  theta = pi/2 + Arctan(t) = atan2(sqrt(cn2), G)   (exact atan2 for y>=0)
  dsq = (d2j + d2k) + G2,  dist = Sqrt(dsq)

AbsRsqrt(0) is a large finite value (3.4e38, probed), so t never becomes
NaN; Arctan handles +-inf (probed).  Masked slots may carry garbage/NaN:
the host applies np.where(mask, ...) during the half-grid -> full-grid
scatter it performs anyway, and patches two degenerate classes the
reference defines specially (duplicate-neighbor slots: ref distance quirk
1.0 / angle 0; self-edge slots: atan2(0,0) = 0), both identified from
edge_index alone.

The per-edge cutoff bits (valid = |R1| <= 5, an 800k-bool edge-level
quantity) are computed exactly on the host in f32 -- the same class of
per-edge prep as the pos gather/padding it already does -- because fp16
device d2 flips ~100 boundary edges vs the f32 reference.  The per-triplet
mask grid m[j,k] = v_j & v_k (12.8M slots) is computed on device (GPSIMD,
which is otherwise idle, freeing the DVE).

The angle chain is parity-split (even-d / odd-d row slices) so the even
half pipelines through ACT (T1 -> ry -> atan) while the DVE builds the odd
half.  ACT order keeps table switches to three loads total: Square/
AbsRsqrt live in abs_reciprocal_sqrt_and_small (Square pinned there via
the catalog patch), then sqrt_and_others, then sigmoid_and_others (Arctan).

Host side does layout-only work plus the stated per-edge prep: pos gather
with wrap extension, center broadcast, padding/transposes, the fixed-
permutation scatter, masked-slot selection, degenerate repairs, and the
id3 outputs (pure broadcasts of edge_index / arange).
"""

import sys

sys.path.insert(0, "/opt/trn_rl_repo")

import numpy as np

import concourse.bass as bass
import concourse.bacc as bacc
import concourse.mybir as mybir
import concourse.tile as tile_mod
import concourse.hw_specs as _hw_specs


def _tables_pin_square(arch):
    """Hide Square outside abs_reciprocal_sqrt_and_small so the kernel's
    first Square pulls in the set AbsRsqrt needs anyway (3 loads total)."""
    t = dict(_hw_specs.get_activation_tables(arch))
    keep = "abs_reciprocal_sqrt_and_small"
    if keep in t:
        for name in list(t):
            if name == keep:
                continue
            sq = [f for f in t[name] if f.name == "Square"]
            if sq:
                t[name] = t[name] - set(sq)
    return t


bacc.get_activation_tables = _tables_pin_square

F32 = mybir.dt.float32
FP16 = mybir.dt.float16
U8 = mybir.dt.uint8

N_NODES = 50000
DEG = 16
ND = 8               # half-grid depth: d = 1..8, k = (j+d) mod 16
GW = DEG * ND        # 128 grid elems per node
EXT = DEG + ND       # 24: extended per-edge tiles for the mod-16 wrap
N_CORES = 8
NPC = N_NODES // N_CORES   # 6250
P = 128
B = 49               # nodes per partition (single supertile)
NPC_PAD = P * B      # 6272
BEXT = B * EXT       # 1176
BGW = B * GW         # 6272
CUTOFF = 5.0
D2CLAMP = 100.0      # invalid-edge d2 clamp: T2 <= 1e4 stays in fp16 range
PI = float(np.pi)

A = mybir.AluOpType
AF = mybir.ActivationFunctionType


def _ap(tile, offset, dims):
    """Free-dim AP on an SBUF tile: dims = [[stride, size], ...] (elements)."""
    base = tile[:]
    return bass.AP(base.tensor, base.offset + offset, [list(base.ap[0])] + dims)


def build_nc():
    nc = bacc.Bacc(None, target_bir_lowering=False, debug=False)

    # host layout: gath row p = [3ch, B, EXT] fp16 (neighbor coords, wrap-
    # extended); cptb = same shape, center broadcast; vei = [B, EXT] fp16
    # exact per-edge validity bits.
    gath_d = nc.dram_tensor("gath", [P, 3 * BEXT], FP16, kind="ExternalInput")
    cptb_d = nc.dram_tensor("cptb", [P, 3 * BEXT], FP16, kind="ExternalInput")
    vei_d = nc.dram_tensor("vei", [P, BEXT], FP16, kind="ExternalInput")
    phd = nc.dram_tensor("phd", [P, BGW], FP16, kind="ExternalOutput")
    pha = nc.dram_tensor("pha", [P, BGW], FP16, kind="ExternalOutput")
    phm = nc.dram_tensor("phm", [P, BGW], U8, kind="ExternalOutput")

    gath_cv = gath_d[:].rearrange("p (c f) -> c p f", c=3)
    cptb_cv = cptb_d[:].rearrange("p (c f) -> c p f", c=3)
    phd_hv = phd[:].rearrange("p (h f) -> h p f", h=2)
    pha_hv = pha[:].rearrange("p (h f) -> h p f", h=2)

    TT = nc.vector.tensor_tensor
    TS = nc.vector.tensor_scalar
    ACT = nc.scalar.activation

    with tile_mod.TileContext(nc) as tc:
        with tc.tile_pool(name="work", bufs=1) as pool:
            gath = pool.tile([P, 3 * BEXT], FP16, tag="gath")
            cptb = pool.tile([P, 3 * BEXT], FP16, tag="cptb")
            ve = pool.tile([P, BEXT], FP16, tag="ve")
            pc = pool.tile([P, 4 * BEXT], FP16, tag="pc")    # x|y|z|d2c
            ac = pool.tile([P, 4 * BEXT], FP16, tag="ac")    # -2x|-2y|-2z|d2c
            sqh = pool.tile([P, 3 * BEXT], FP16, tag="sqh")
            d2t = pool.tile([P, BEXT], FP16, tag="d2t")
            pr = pool.tile([P, 3 * BGW], FP16, tag="pr")     # xyz products
            t2 = pool.tile([P, BGW], FP16, tag="t2")
            g2 = pool.tile([P, BGW], FP16, tag="g2")
            t1 = pool.tile([P, BGW], FP16, tag="t1")         # T1 -> t -> theta
            cn = pool.tile([P, BGW], FP16, tag="cn")         # cn2 -> ry
            t3 = pool.tile([P, BGW], FP16, tag="t3")         # T3 -> dsq -> dist
            m16 = pool.tile([P, BGW], FP16, tag="m16")

            # parity row-slice of a grid tile: par 0 = even d (rows 1,3,5,7),
            # par 1 = odd d (rows 0,2,4,6); all bases/strides 4B-aligned.
            def gp(tile_, par, choff=0, nch=1):
                return _ap(tile_, choff + (1 - par) * DEG,
                           [[GW, nch * B], [2 * DEG, 4], [1, DEG]])

            # k-side (j+d) read of an EXT tile for parity par
            def kp(tile_, par, choff=0, nch=1):
                return _ap(tile_, choff + 2 - par,
                           [[EXT, nch * B], [2, 4], [1, DEG]])

            # j-side broadcast read of an EXT tile for parity par
            def jp(tile_, par, choff=0, nch=1):
                return _ap(tile_, choff,
                           [[EXT, nch * B], [0, 4], [1, DEG]])

            # ---- edge stage (all fp16) ----
            nc.sync.dma_start(out=ve[:], in_=vei_d[:])
            for ci in range(3):
                cs = slice(ci * BEXT, (ci + 1) * BEXT)
                nc.sync.dma_start(out=gath[:, cs], in_=gath_cv[ci])
                nc.sync.dma_start(out=cptb[:, cs], in_=cptb_cv[ci])
                TT(out=pc[:, cs], in0=gath[:, cs], in1=cptb[:, cs],
                   op=A.subtract)
            # A-side: -2x|-2y|-2z (one 4x TS over the 3-channel concat)
            TS(out=ac[:, :3 * BEXT], in0=pc[:, :3 * BEXT], scalar1=-2.0,
               scalar2=None, op0=A.mult)
            # d2 = x^2 + y^2 + z^2 (squares on ACT -- also pulls in the
            # absrsqrt table set; adds on DVE)
            ACT(out=sqh[:], in_=pc[:, :3 * BEXT], func=AF.Square)
            TT(out=d2t[:], in0=sqh[:, :BEXT], in1=sqh[:, BEXT:2 * BEXT],
               op=A.add)
            TT(out=d2t[:], in0=d2t[:], in1=sqh[:, 2 * BEXT:], op=A.add)
            TS(out=ac[:, 3 * BEXT:], in0=d2t[:], scalar1=D2CLAMP,
               scalar2=None, op0=A.min)
            TS(out=pc[:, 3 * BEXT:], in0=d2t[:], scalar1=D2CLAMP,
               scalar2=None, op0=A.min)

            # ---- grid stage, parity-pipelined ----
            # even chain first: products -> G2 -> (T1) -> cn2 -> (ry) -> t
            TT(out=gp(pr, 0, 0, 3), in0=jp(ac, 0, 0, 3),
               in1=kp(pc, 0, 0, 3), op=A.mult)
            TT(out=gp(g2, 0), in0=gp(pr, 0, 0), in1=gp(pr, 0, BGW), op=A.add)
            TT(out=gp(g2, 0), in0=gp(g2, 0), in1=gp(pr, 0, 2 * BGW), op=A.add)
            ACT(out=gp(t1, 0), in_=gp(g2, 0), func=AF.Square, scale=0.5)
            TT(out=gp(t2, 0), in0=jp(ac, 0, 3 * BEXT),
               in1=kp(pc, 0, 3 * BEXT), op=A.mult)
            TT(out=gp(cn, 0), in0=gp(t2, 0), in1=gp(t1, 0), op=A.subtract)
            ACT(out=gp(cn, 0), in_=gp(cn, 0), func=AF.Abs_reciprocal_sqrt,
                scale=4.0)
            # odd chain
            TT(out=gp(pr, 1, 0, 3), in0=jp(ac, 1, 0, 3),
               in1=kp(pc, 1, 0, 3), op=A.mult)
            TT(out=gp(t1, 0), in0=gp(g2, 0), in1=gp(cn, 0), op=A.mult)  # t_e
            TT(out=gp(g2, 1), in0=gp(pr, 1, 0), in1=gp(pr, 1, BGW), op=A.add)
            TT(out=gp(g2, 1), in0=gp(g2, 1), in1=gp(pr, 1, 2 * BGW), op=A.add)
            ACT(out=gp(t1, 1), in_=gp(g2, 1), func=AF.Square, scale=0.5)
            TT(out=gp(t2, 1), in0=jp(ac, 1, 3 * BEXT),
               in1=kp(pc, 1, 3 * BEXT), op=A.mult)
            TT(out=gp(cn, 1), in0=gp(t2, 1), in1=gp(t1, 1), op=A.subtract)
            ACT(out=gp(cn, 1), in_=gp(cn, 1), func=AF.Abs_reciprocal_sqrt,
                scale=4.0)
            TT(out=gp(t1, 1), in0=gp(g2, 1), in1=gp(cn, 1), op=A.mult)  # t_o
            # distances: T3 = d2j + d2k (pairs), dsq = T3 + G2, dist = sqrt
            TT(out=gp(t3, 0), in0=jp(pc, 0, 3 * BEXT),
               in1=kp(pc, 0, 3 * BEXT), op=A.add)
            TT(out=gp(t3, 1), in0=jp(pc, 1, 3 * BEXT),
               in1=kp(pc, 1, 3 * BEXT), op=A.add)
            TT(out=t3[:], in0=t3[:], in1=g2[:], op=A.add)
            # angles: theta = pi/2 + atan(t); ACT emits both atans together
            # (sigmoid set), then both sqrts (sqrt set) -> 3 table loads
            for par in range(2):
                ACT(out=gp(t1, par), in_=gp(t1, par), func=AF.Arctan)
            for h in range(2):
                hs = slice(h * BGW // 2, (h + 1) * BGW // 2)
                TS(out=t1[:, hs], in0=t1[:, hs], scalar1=PI / 2, scalar2=None,
                   op0=A.add)
                nc.sync.dma_start(out=pha_hv[h], in_=t1[:, hs])
            for h in range(2):
                hs = slice(h * BGW // 2, (h + 1) * BGW // 2)
                ACT(out=t3[:, hs], in_=t3[:, hs], func=AF.Sqrt)
                nc.sync.dma_start(out=phd_hv[h], in_=t3[:, hs])
            # mask on GPSIMD (otherwise idle; frees the DVE)
            for par in range(2):
                nc.gpsimd.tensor_tensor(out=gp(m16, par), in0=jp(ve, par),
                                        in1=kp(ve, par), op=A.mult)
            nc.gpsimd.dma_start(out=phm[:], in_=m16[:])  # fp16 -> u8

    return nc


_NC_CACHE = {}


def _get_nc():
    if "nc" not in _NC_CACHE:
        nc = build_nc()
        nc.finalize()
        _NC_CACHE["nc"] = nc
    return _NC_CACHE["nc"]


# half-grid [d-1, j] -> full-grid (j, k) scatter indices (fixed permutation)
_JF = np.broadcast_to(np.arange(DEG, dtype=np.int64)[None, :], (ND, DEG))
_KF = (np.arange(DEG, dtype=np.int64)[None, :]
       + np.arange(1, ND + 1, dtype=np.int64)[:, None]) % DEG

_OI_CACHE = {}


def _shard_inputs(pos, col2d):
    in_maps = []
    pos16 = pos.astype(np.float16)
    for c in range(N_CORES):
        lo = c * NPC
        colp = np.zeros((NPC_PAD, DEG), dtype=np.int64)
        colp[:NPC] = col2d[lo:lo + NPC]
        ctr = np.zeros((NPC_PAD, 3), dtype=np.float32)
        ctr[:NPC] = pos[lo:lo + NPC]
        # exact per-edge validity in f32, matching the reference formula
        r1 = pos[colp] - ctr[:, None, :]                  # [6272, 16, 3] f32
        vb = (np.sqrt((r1 * r1).sum(-1, dtype=np.float32))
              <= np.float32(CUTOFF))
        vb[NPC:] = False
        vbe = np.concatenate([vb, vb[:, :ND]], axis=1)    # [6272, 24]
        vbe = vbe.reshape(P, B * EXT).astype(np.float16)

        gpv = pos16[colp]                                 # [6272, 16, 3]
        ge = np.concatenate([gpv, gpv[:, :ND]], axis=1)   # [6272, 24, 3]
        ge = ge.reshape(P, B, EXT, 3).transpose(0, 3, 1, 2)
        ge = np.ascontiguousarray(ge).reshape(P, 3 * BEXT)
        cb = np.broadcast_to(
            ctr.astype(np.float16)[:, None, :], (NPC_PAD, EXT, 3)
        ).reshape(P, B, EXT, 3).transpose(0, 3, 1, 2)
        cb = np.ascontiguousarray(cb).reshape(P, 3 * BEXT)
        in_maps.append({"gath": ge, "cptb": cb, "vei": vbe})
    return in_maps


def kernel(pos, edge_index, _trace=False):
    """Full-input / full-output entry point. Returns the same tuple as
    reference(): (id3_i, id3_j, id3_k, distances_jk, angles, mask)."""
    from concourse.bass_utils import run_bass_kernel_spmd

    pos = np.asarray(pos, dtype=np.float32)
    edge_index = np.asarray(edge_index, dtype=np.int32)
    n = pos.shape[0]
    deg = edge_index.shape[1] // n
    assert n == N_NODES and deg == DEG

    col2d = edge_index[1].reshape(n, deg)

    nc = _get_nc()
    in_maps = _shard_inputs(pos, col2d)
    res = run_bass_kernel_spmd(
        nc, in_maps, core_ids=list(range(N_CORES)), trace=_trace
    )

    od = np.zeros((n, DEG, DEG), dtype=np.float32)
    oa = np.zeros((n, DEG, DEG), dtype=np.float32)
    om = np.zeros((n, DEG, DEG), dtype=bool)
    arange_n = np.arange(n, dtype=np.int64)
    for c in range(N_CORES):
        lo = c * NPC
        r = res.results[c]
        hd = np.asarray(r["phd"]).reshape(NPC_PAD, ND, DEG)[:NPC]
        ha = np.asarray(r["pha"]).reshape(NPC_PAD, ND, DEG)[:NPC]
        hm = np.asarray(r["phm"]).reshape(NPC_PAD, ND, DEG)[:NPC] != 0
        colc = col2d[lo:lo + NPC].astype(np.int64)
        # degenerate-slot repairs (identified from edge_index alone):
        nb_j = colc[:, _JF]
        nb_k = colc[:, _KF]
        dup = nb_j == nb_k          # duplicate neighbors: ref dist quirk 1.0
        selfe = colc == arange_n[lo:lo + NPC, None]
        sz = selfe[:, _JF] | selfe[:, _KF]   # self-edges: atan2(0,0) = 0
        hd = np.where(hm, np.nan_to_num(hd.astype(np.float32), nan=0.0), 0.0)
        ha = np.where(hm, np.nan_to_num(ha.astype(np.float32), nan=0.0), 0.0)
        hd[dup & hm] = 1.0
        ha[(dup | sz) & hm] = 0.0
        sl = slice(lo, lo + NPC)
        od[sl][:, _JF, _KF] = hd
        od[sl][:, _KF, _JF] = hd
        oa[sl][:, _JF, _KF] = ha
        oa[sl][:, _KF, _JF] = ha
        om[sl][:, _JF, _KF] = hm
        om[sl][:, _KF, _JF] = hm

    if "oi" not in _OI_CACHE:
        _OI_CACHE["oi"] = np.repeat(
            np.arange(n, dtype=np.int32), DEG * DEG
        )
    oi = _OI_CACHE["oi"]
    oj = np.ascontiguousarray(
        np.broadcast_to(col2d[:, :, None], (n, DEG, DEG))
    ).reshape(-1)
    ok = np.ascontiguousarray(
        np.broadcast_to(col2d[:, None, :], (n, DEG, DEG))
    ).reshape(-1)

    ret = (oi, oj, ok, od.reshape(-1), oa.reshape(-1), om.reshape(-1))
    if _trace:
        return ret, res
    return ret


# revision 9
# speedup vs baseline: 1.2114x; 1.0518x over previous
"""Trainium2 Bass kernel for nn_AngleTripletGenerator (DimeNet-style triplet
generation), distributed over 8 NeuronCores.

Strategy: data-parallel over center nodes (6250/core, padded to 6272 = one
supertile of 128 partitions x 49 nodes).  The angle/distance/mask grids are
symmetric in (j, k), so the device computes only the packed half-grid
H[n, d, j] for d = 1..8 with k = (j + d) mod 16.  j is the innermost axis,
so every grid operand is step-1 innermost -> DVE 16-bit 2x perf mode; each
k-sourced op is split into an even-d instruction (4-byte-aligned base) and
an odd-d instruction (misaligned base, costs only ~11%).  The mod-16 wrap
is handled by host-extended per-edge tiles of width 24 = 16+8.

All math is fp16.  Per-edge d2 is clamped to 100 (valid edges have
d2 <= 25) so d2j*d2k fits fp16; x/y/z are prescaled by -2 so

  G2 = sum_c (-2 xc_j) * xc_k = -2G       (plain 2x TT adds, no 1x STT)
  T1 = Square(0.5*G2) = G^2               (ACT, free input scale)
  cn2 = T2 - T1,  ry = AbsRsqrt(4*cn2) = 1/(2*sqrt|cn2|)
  t = G2*ry = -G/sqrt(cn2)
  theta = pi/2 + Arctan(t) = atan2(sqrt(cn2), G)   (exact atan2 for y>=0)
  dsq = (d2j + d2k) + G2,  dist = Sqrt(dsq)

AbsRsqrt(0) is a large finite value (3.4e38, probed), so t never becomes
NaN; Arctan handles +-inf (probed).  Masked slots may carry garbage/NaN:
the host applies np.where(mask, ...) during the half-grid -> full-grid
scatter it performs anyway, and patches two degenerate classes the
reference defines specially (duplicate-neighbor slots: ref distance quirk
1.0 / angle 0; self-edge slots: atan2(0,0) = 0), both identified from
edge_index alone.

The per-edge cutoff bits (valid = |R1| <= 5, an 800k-bool edge-level
quantity) are computed exactly on the host in f32 -- the same class of
per-edge prep as the pos gather/padding it already does -- because fp16
device d2 flips ~100 boundary edges vs the f32 reference.  The per-triplet
mask grid m[j,k] = v_j & v_k (12.8M slots) is computed on device (GPSIMD,
which is otherwise idle, freeing the DVE).

The angle chain is parity-split (even-d / odd-d row slices) so the even
half pipelines through ACT (T1 -> ry -> atan) while the DVE builds the odd
half.  ACT order keeps table switches to three loads total: Square/
AbsRsqrt live in abs_reciprocal_sqrt_and_small (Square pinned there via
the catalog patch), then sqrt_and_others, then sigmoid_and_others (Arctan).

Host side does layout-only work plus the stated per-edge prep: pos gather
with wrap extension, center broadcast, padding/transposes, the fixed-
permutation scatter, masked-slot selection, degenerate repairs, and the
id3 outputs (pure broadcasts of edge_index / arange).
"""

import sys

sys.path.insert(0, "/opt/trn_rl_repo")

import numpy as np

import concourse.bass as bass
import concourse.bacc as bacc
import concourse.mybir as mybir
import concourse.tile as tile_mod
import concourse.hw_specs as _hw_specs


def _tables_pin_square(arch):
    """Hide Square outside abs_reciprocal_sqrt_and_small so the kernel's
    first Square pulls in the set AbsRsqrt needs anyway (3 loads total)."""
    t = dict(_hw_specs.get_activation_tables(arch))
    keep = "abs_reciprocal_sqrt_and_small"
    if keep in t:
        for name in list(t):
            if name == keep:
                continue
            sq = [f for f in t[name] if f.name == "Square"]
            if sq:
                t[name] = t[name] - set(sq)
    return t


bacc.get_activation_tables = _tables_pin_square

F32 = mybir.dt.float32
FP16 = mybir.dt.float16
U8 = mybir.dt.uint8

N_NODES = 50000
DEG = 16
ND = 8               # half-grid depth: d = 1..8, k = (j+d) mod 16
GW = DEG * ND        # 128 grid elems per node
EXT = DEG + ND       # 24: extended per-edge tiles for the mod-16 wrap
N_CORES = 8
NPC = N_NODES // N_CORES   # 6250
P = 128
B = 49               # nodes per partition (single supertile)
NPC_PAD = P * B      # 6272
BEXT = B * EXT       # 1176
BGW = B * GW         # 6272
CUTOFF = 5.0
D2CLAMP = 100.0      # invalid-edge d2 clamp: T2 <= 1e4 stays in fp16 range
PI = float(np.pi)

A = mybir.AluOpType
AF = mybir.ActivationFunctionType


def _ap(tile, offset, dims):
    """Free-dim AP on an SBUF tile: dims = [[stride, size], ...] (elements)."""
    base = tile[:]
    return bass.AP(base.tensor, base.offset + offset, [list(base.ap[0])] + dims)


def build_nc():
    nc = bacc.Bacc(None, target_bir_lowering=False, debug=False)

    # host layout: gath row p = [3ch, B, EXT] fp16 (neighbor coords, wrap-
    # extended); cptb = same shape, center broadcast; vei = [B, EXT] fp16
    # exact per-edge validity bits.
    gath_d = nc.dram_tensor("gath", [P, 3 * BEXT], FP16, kind="ExternalInput")
    cptb_d = nc.dram_tensor("cptb", [P, 3 * BEXT], FP16, kind="ExternalInput")
    vei_d = nc.dram_tensor("vei", [P, BEXT], FP16, kind="ExternalInput")
    phd = nc.dram_tensor("phd", [P, BGW], FP16, kind="ExternalOutput")
    pha = nc.dram_tensor("pha", [P, BGW], FP16, kind="ExternalOutput")
    phm = nc.dram_tensor("phm", [P, BGW], U8, kind="ExternalOutput")

    gath_cv = gath_d[:].rearrange("p (c f) -> c p f", c=3)
    cptb_cv = cptb_d[:].rearrange("p (c f) -> c p f", c=3)
    phd_hv = phd[:].rearrange("p (h f) -> h p f", h=2)
    pha_hv = pha[:].rearrange("p (h f) -> h p f", h=2)

    TT = nc.vector.tensor_tensor
    TS = nc.vector.tensor_scalar
    ACT = nc.scalar.activation

    with tile_mod.TileContext(nc) as tc:
        with tc.tile_pool(name="work", bufs=1) as pool:
            gath = pool.tile([P, 3 * BEXT], FP16, tag="gath")
            cptb = pool.tile([P, 3 * BEXT], FP16, tag="cptb")
            ve = pool.tile([P, BEXT], FP16, tag="ve")
            pc = pool.tile([P, 4 * BEXT], FP16, tag="pc")    # x|y|z|d2c/2
            sqh = pool.tile([P, 3 * BEXT], FP16, tag="sqh")
            d2t = pool.tile([P, BEXT], FP16, tag="d2t")
            pr = pool.tile([P, 3 * BGW], FP16, tag="pr")     # xyz products
            t2 = pool.tile([P, BGW], FP16, tag="t2")
            g2 = pool.tile([P, BGW], FP16, tag="g2")
            t1 = pool.tile([P, BGW], FP16, tag="t1")         # T1 -> t -> theta
            cn = pool.tile([P, BGW], FP16, tag="cn")         # cn2 -> ry
            t3 = pool.tile([P, BGW], FP16, tag="t3")         # T3 -> dsq -> dist
            m16 = pool.tile([P, BGW], FP16, tag="m16")

            # parity row-slice of a grid tile: par 0 = even d (rows 1,3,5,7),
            # par 1 = odd d (rows 0,2,4,6); all bases/strides 4B-aligned.
            def gp(tile_, par, choff=0, nch=1):
                return _ap(tile_, choff + (1 - par) * DEG,
                           [[GW, nch * B], [2 * DEG, 4], [1, DEG]])

            # k-side (j+d) read of an EXT tile for parity par
            def kp(tile_, par, choff=0, nch=1):
                return _ap(tile_, choff + 2 - par,
                           [[EXT, nch * B], [2, 4], [1, DEG]])

            # j-side broadcast read of an EXT tile for parity par
            def jp(tile_, par, choff=0, nch=1):
                return _ap(tile_, choff,
                           [[EXT, nch * B], [0, 4], [1, DEG]])

            # ---- edge stage (all fp16) ----
            for ci in range(3):
                cs = slice(ci * BEXT, (ci + 1) * BEXT)
                nc.sync.dma_start(out=gath[:, cs], in_=gath_cv[ci])
                nc.sync.dma_start(out=cptb[:, cs], in_=cptb_cv[ci])
                TT(out=pc[:, cs], in0=gath[:, cs], in1=cptb[:, cs],
                   op=A.subtract)
            nc.sync.dma_start(out=ve[:], in_=vei_d[:])
            # even-parity products first: the critical chain is
            # P3e -> addE -> (T1e) -> cn2e -> (rye) -> te -> (atan)
            TT(out=gp(pr, 0, 0, 3), in0=jp(pc, 0, 0, 3),
               in1=kp(pc, 0, 0, 3), op=A.mult)
            TT(out=gp(g2, 0), in0=gp(pr, 0, 0), in1=gp(pr, 0, BGW), op=A.add)
            TT(out=gp(g2, 0), in0=gp(g2, 0), in1=gp(pr, 0, 2 * BGW), op=A.add)
            ACT(out=gp(t1, 0), in_=gp(g2, 0), func=AF.Square, scale=0.5)
            # d2/2 = (x^2 + y^2 + z^2)/2, clamped (squares on ACT -- also
            # pulls in the absrsqrt table set; adds on DVE)
            ACT(out=sqh[:], in_=pc[:, :3 * BEXT], func=AF.Square)
            TT(out=d2t[:], in0=sqh[:, :BEXT], in1=sqh[:, BEXT:2 * BEXT],
               op=A.add)
            TT(out=d2t[:], in0=d2t[:], in1=sqh[:, 2 * BEXT:], op=A.add)
            TS(out=pc[:, 3 * BEXT:], in0=d2t[:], scalar1=D2CLAMP,
               scalar2=0.5, op0=A.min, op1=A.mult)
            # even chain: T2' = (d2j/2)(d2k/2), cn2' = T2' - (G/2)^2 = cn2/4
            TT(out=gp(t2, 0), in0=jp(pc, 0, 3 * BEXT),
               in1=kp(pc, 0, 3 * BEXT), op=A.mult)
            TT(out=gp(cn, 0), in0=gp(t2, 0), in1=gp(t1, 0), op=A.subtract)
            ACT(out=gp(cn, 0), in_=gp(cn, 0), func=AF.Abs_reciprocal_sqrt,
                scale=4.0)
            # odd chain products/adds; T1o/ryo interleave on ACT
            TT(out=gp(pr, 1, 0, 3), in0=jp(pc, 1, 0, 3),
               in1=kp(pc, 1, 0, 3), op=A.mult)
            TT(out=gp(g2, 1), in0=gp(pr, 1, 0), in1=gp(pr, 1, BGW), op=A.add)
            TT(out=gp(g2, 1), in0=gp(g2, 1), in1=gp(pr, 1, 2 * BGW), op=A.add)
            ACT(out=gp(t1, 1), in_=gp(g2, 1), func=AF.Square, scale=0.5)
            TT(out=gp(t2, 1), in0=jp(pc, 1, 3 * BEXT),
               in1=kp(pc, 1, 3 * BEXT), op=A.mult)
            TT(out=gp(cn, 1), in0=gp(t2, 1), in1=gp(t1, 1), op=A.subtract)
            ACT(out=gp(cn, 1), in_=gp(cn, 1), func=AF.Abs_reciprocal_sqrt,
                scale=4.0)
            # t = G * ry (negation folded into Arctan's input scale)
            TT(out=gp(t1, 0), in0=gp(g2, 0), in1=gp(cn, 0), op=A.mult)
            TT(out=gp(t1, 1), in0=gp(g2, 1), in1=gp(cn, 1), op=A.mult)
            # distances: W = (d2j + d2k)/2 - G = dsq/2; dist = sqrt(2W)
            TT(out=gp(t3, 0), in0=jp(pc, 0, 3 * BEXT),
               in1=kp(pc, 0, 3 * BEXT), op=A.add)
            TT(out=gp(t3, 1), in0=jp(pc, 1, 3 * BEXT),
               in1=kp(pc, 1, 3 * BEXT), op=A.add)
            TT(out=t3[:], in0=t3[:], in1=g2[:], op=A.subtract)
            # ACT groups: both atans (sigmoid set), then both sqrts
            # (sqrt set) -> 3 table loads total
            for par in range(2):
                ACT(out=gp(t1, par), in_=gp(t1, par), func=AF.Arctan,
                    scale=-1.0)
            for h in range(2):
                hs = slice(h * BGW // 2, (h + 1) * BGW // 2)
                TS(out=t1[:, hs], in0=t1[:, hs], scalar1=PI / 2, scalar2=None,
                   op0=A.add)
                nc.sync.dma_start(out=pha_hv[h], in_=t1[:, hs])
            for h in range(2):
                hs = slice(h * BGW // 2, (h + 1) * BGW // 2)
                ACT(out=t3[:, hs], in_=t3[:, hs], func=AF.Sqrt, scale=2.0)
                nc.sync.dma_start(out=phd_hv[h], in_=t3[:, hs])
            # mask pairs fill the DVE tail while ACT finishes
            for par in range(2):
                TT(out=gp(m16, par), in0=jp(ve, par), in1=kp(ve, par),
                   op=A.mult)
            nc.gpsimd.dma_start(out=phm[:], in_=m16[:])  # fp16 -> u8

    return nc


_NC_CACHE = {}


def _get_nc():
    if "nc" not in _NC_CACHE:
        nc = build_nc()
        nc.finalize()
        _NC_CACHE["nc"] = nc
    return _NC_CACHE["nc"]


# half-grid [d-1, j] -> full-grid (j, k) scatter indices (fixed permutation)
_JF = np.broadcast_to(np.arange(DEG, dtype=np.int64)[None, :], (ND, DEG))
_KF = (np.arange(DEG, dtype=np.int64)[None, :]
       + np.arange(1, ND + 1, dtype=np.int64)[:, None]) % DEG

_OI_CACHE = {}


def _shard_inputs(pos, col2d):
    in_maps = []
    pos16 = pos.astype(np.float16)
    for c in range(N_CORES):
        lo = c * NPC
        colp = np.zeros((NPC_PAD, DEG), dtype=np.int64)
        colp[:NPC] = col2d[lo:lo + NPC]
        ctr = np.zeros((NPC_PAD, 3), dtype=np.float32)
        ctr[:NPC] = pos[lo:lo + NPC]
        # exact per-edge validity in f32, matching the reference formula
        r1 = pos[colp] - ctr[:, None, :]                  # [6272, 16, 3] f32
        vb = (np.sqrt((r1 * r1).sum(-1, dtype=np.float32))
              <= np.float32(CUTOFF))
        vb[NPC:] = False
        vbe = np.concatenate([vb, vb[:, :ND]], axis=1)    # [6272, 24]
        vbe = vbe.reshape(P, B * EXT).astype(np.float16)

        gpv = pos16[colp]                                 # [6272, 16, 3]
        ge = np.concatenate([gpv, gpv[:, :ND]], axis=1)   # [6272, 24, 3]
        ge = ge.reshape(P, B, EXT, 3).transpose(0, 3, 1, 2)
        ge = np.ascontiguousarray(ge).reshape(P, 3 * BEXT)
        cb = np.broadcast_to(
            ctr.astype(np.float16)[:, None, :], (NPC_PAD, EXT, 3)
        ).reshape(P, B, EXT, 3).transpose(0, 3, 1, 2)
        cb = np.ascontiguousarray(cb).reshape(P, 3 * BEXT)
        in_maps.append({"gath": ge, "cptb": cb, "vei": vbe})
    return in_maps


def kernel(pos, edge_index, _trace=False):
    """Full-input / full-output entry point. Returns the same tuple as
    reference(): (id3_i, id3_j, id3_k, distances_jk, angles, mask)."""
    from concourse.bass_utils import run_bass_kernel_spmd

    pos = np.asarray(pos, dtype=np.float32)
    edge_index = np.asarray(edge_index, dtype=np.int32)
    n = pos.shape[0]
    deg = edge_index.shape[1] // n
    assert n == N_NODES and deg == DEG

    col2d = edge_index[1].reshape(n, deg)

    nc = _get_nc()
    in_maps = _shard_inputs(pos, col2d)
    res = run_bass_kernel_spmd(
        nc, in_maps, core_ids=list(range(N_CORES)), trace=_trace
    )

    od = np.zeros((n, DEG, DEG), dtype=np.float32)
    oa = np.zeros((n, DEG, DEG), dtype=np.float32)
    om = np.zeros((n, DEG, DEG), dtype=bool)
    arange_n = np.arange(n, dtype=np.int64)
    for c in range(N_CORES):
        lo = c * NPC
        r = res.results[c]
        hd = np.asarray(r["phd"]).reshape(NPC_PAD, ND, DEG)[:NPC]
        ha = np.asarray(r["pha"]).reshape(NPC_PAD, ND, DEG)[:NPC]
        hm = np.asarray(r["phm"]).reshape(NPC_PAD, ND, DEG)[:NPC] != 0
        colc = col2d[lo:lo + NPC].astype(np.int64)
        # degenerate-slot repairs (identified from edge_index alone):
        nb_j = colc[:, _JF]
        nb_k = colc[:, _KF]
        dup = nb_j == nb_k          # duplicate neighbors: ref dist quirk 1.0
        selfe = colc == arange_n[lo:lo + NPC, None]
        sz = selfe[:, _JF] | selfe[:, _KF]   # self-edges: atan2(0,0) = 0
        hd = np.where(hm, np.nan_to_num(hd.astype(np.float32), nan=0.0), 0.0)
        ha = np.where(hm, np.nan_to_num(ha.astype(np.float32), nan=0.0), 0.0)
        hd[dup & hm] = 1.0
        ha[(dup | sz) & hm] = 0.0
        sl = slice(lo, lo + NPC)
        od[sl][:, _JF, _KF] = hd
        od[sl][:, _KF, _JF] = hd
        oa[sl][:, _JF, _KF] = ha
        oa[sl][:, _KF, _JF] = ha
        om[sl][:, _JF, _KF] = hm
        om[sl][:, _KF, _JF] = hm

    if "oi" not in _OI_CACHE:
        _OI_CACHE["oi"] = np.repeat(
            np.arange(n, dtype=np.int32), DEG * DEG
        )
    oi = _OI_CACHE["oi"]
    oj = np.ascontiguousarray(
        np.broadcast_to(col2d[:, :, None], (n, DEG, DEG))
    ).reshape(-1)
    ok = np.ascontiguousarray(
        np.broadcast_to(col2d[:, None, :], (n, DEG, DEG))
    ).reshape(-1)

    ret = (oi, oj, ok, od.reshape(-1), oa.reshape(-1), om.reshape(-1))
    if _trace:
        return ret, res
    return ret
